# revision 6
# baseline (speedup 1.0000x reference)
"""Biased self-attention layer (graph-batched) on 8 Trainium2 NeuronCores.

Data-parallel over B=64 graphs, 8 graphs/core, dense-padded to N=256.

v2 design (vs baseline): single fused pass per graph (QKV -> attention ->
out-proj -> LN) with no DRAM scratch roundtrip; bf16 matmul operands (f32
PSUM); softmax computed as exp(S) * exp(bias) with host-precomputed
exp-bias (masked keys become exact 0), so the bias "add" is a bf16
multiply on the otherwise-idle GPSIMD engine; per-query softmax
normalization via DVE reciprocal of the denominator row + PE broadcast
matmul; work emitted on a software-pipelined slot schedule that keeps the
tensor engine continuously busy (3 graph stages in flight: QKV(g),
attention(g-1), out-proj/LN(g-2)).
"""

import sys

for _p in ("/opt/pypackages", "/opt/trn_rl_repo"):
    if _p not in sys.path:
        sys.path.insert(0, _p)

import numpy as np

B, N, H, D = 64, 256, 16, 1024
DH = D // H
SCALE = DH ** -0.5
EPS = 1e-5
NCORES = 8
GPC = B // NCORES          # graphs per core
RPC = GPC * N              # padded rows per core (2048)

_CACHE = {}


def _split_excess_waits(nc, maxw=1):
    """This walrus build accepts at most one sync-wait per instruction
    (TPB_CTRL and S3_LW structs reject more). Move excess waits onto
    preceding single-wait NOPs on the same engine queue."""
    import bass_rust

    fn = nc.m.functions[0]
    for bb in fn.blocks:
        insts = bb.instructions
        out = []
        for inst in list(insts):
            si = inst.sync_info
            if si is not None and si.on_wait and len(si.on_wait) > maxw:
                waits = list(si.on_wait)
                si.on_wait = waits[-maxw:]
                for w in waits[:-maxw]:
                    nop_bi = nc.engines[inst.engine].nop(
                        nofuse=True, hint="wait_split"
                    )
                    ni = nop_bi.ins
                    cur = nc.cur_bb.bb.instructions
                    assert cur[-1].name == ni.name
                    cur.pop()
                    nsi = ni.sync_info
                    if nsi is None:
                        ni.sync_info = bass_rust.SyncInfo(
                            on_wait=[w], on_update=[]
                        )
                    else:
                        nsi.on_wait = [w]
                    out.append(ni)
            out.append(inst)
        if len(out) != len(insts):
            insts[:] = out


def build_program(trivial_gb=True, cs=(N,) * GPC):
    """Build the SPMD Bass program. cs[s] = max valid node count of the
    graphs placed in slot s (graphs are count-sorted on the host so all
    cores share one program); per-head softmax work is trimmed to cs[s]
    query columns, with out-of-range columns left stale-but-finite and
    discarded by the host gather."""
    key = ("nc", trivial_gb, tuple(cs))
    if key in _CACHE:
        return _CACHE[key]

    import concourse.bass as bass
    import concourse.mybir as mybir
    import concourse.tile as tile

    dt = mybir.dt
    AF = mybir.ActivationFunctionType
    f32 = dt.float32
    f32r = dt.float32r
    bf16 = dt.bfloat16

    nc = bass.Bass("TRN2", target_bir_lowering=False, debug=False)

    f8 = dt.float8e4
    DR = mybir.MatmulPerfMode.DoubleRow
    xT = nc.dram_tensor("xT", [D, RPC], f8, kind="ExternalInput")
    xr = nc.dram_tensor("xr", [RPC, D], bf16, kind="ExternalInput")  # x + bp
    ebias = nc.dram_tensor("ebias", [GPC, H, 2, 128, N], bf16,
                           kind="ExternalInput")  # exp(bias^T), masked=0
    wq = nc.dram_tensor("wq", [D, D], f8, kind="ExternalInput")  # Wq.T*scale
    wk = nc.dram_tensor("wk", [D, D], f8, kind="ExternalInput")
    wv = nc.dram_tensor("wv", [D, D], f8, kind="ExternalInput")
    wp = nc.dram_tensor("wp", [D, D], f8, kind="ExternalInput")
    bq_d = nc.dram_tensor("bq_d", [1, 8, 128], f8, kind="ExternalInput")
    bk_d = nc.dram_tensor("bk_d", [1, 8, 128], f8, kind="ExternalInput")
    bvb = nc.dram_tensor("bvb", [128, D], f32, kind="ExternalInput")
    gb = nc.dram_tensor("gb", [128, D], f32, kind="ExternalInput")
    bb = nc.dram_tensor("bb", [128, D], f32, kind="ExternalInput")
    ident = nc.dram_tensor("ident", [128, 128], bf16, kind="ExternalInput")
    y = nc.dram_tensor("y", [RPC, D], bf16, kind="ExternalOutput")

    xT_r = xT.rearrange("(b t p) r -> p b t r", p=128, t=2)
    eb_r = ebias.rearrange("g h jb p i -> p g h jb i")
    xr_r = xr.rearrange("(g ih p) o -> p g ih o", p=128, ih=2)
    y_r = y.rearrange("(g ih p) o -> p g ih o", p=128, ih=2)

    with tile.TileContext(nc) as tc:
        with tc.tile_pool(name="w", bufs=1) as wpool, \
             tc.tile_pool(name="sm", bufs=1) as spool, \
             tc.tile_pool(name="x", bufs=2) as xpool, \
             tc.tile_pool(name="qk", bufs=2) as qkpool, \
             tc.tile_pool(name="v", bufs=2) as vpool, \
             tc.tile_pool(name="eb", bufs=6) as ebpool, \
             tc.tile_pool(name="pt", bufs=4) as ptpool, \
             tc.tile_pool(name="rc", bufs=4) as rcpool, \
             tc.tile_pool(name="ot", bufs=2) as otpool, \
             tc.tile_pool(name="xy", bufs=2) as xypool, \
             tc.tile_pool(name="ln", bufs=4) as lnpool, \
             tc.tile_pool(name="pqk", bufs=3, space="PSUM") as pqkpool, \
             tc.tile_pool(name="pst", bufs=2, space="PSUM") as pstpool, \
             tc.tile_pool(name="pav", bufs=2, space="PSUM") as pavpool, \
             tc.tile_pool(name="ptr", bufs=1, space="PSUM") as ptrpool:

            # ---------------- resident weights / constants ----------------
            # DMA order matters at startup: the first Q matmuls need only
            # wq + x(0) + bq, so issue those first; K/V/P weights stream in
            # while Q projections already run.
            wq_s = wpool.tile([128, 4, 2, D], f8, tag="wq", name="wq")
            wk_s = wpool.tile([128, 4, 2, D], f8, tag="wk", name="wk")
            wv_s = wpool.tile([128, 4, 2, D], f8, tag="wv", name="wv")
            wp_s = wpool.tile([128, 4, 2, D], f8, tag="wp", name="wp")
            bq_s = spool.tile([1, 8, 128], f8, tag="bq", name="bq")
            bk_s = spool.tile([1, 8, 128], f8, tag="bk", name="bk")
            onesr = spool.tile([1, N], f8, tag="onesr", name="onesr")
            nc.vector.memset(onesr[:], 1.0)
            bvb_s = spool.tile([128, D], f32, tag="bvb", name="bvb")

            def w_dma(w_s, w_d):
                nc.sync.dma_start(
                    out=w_s[:],
                    in_=w_d.rearrange("(b t p) o -> p b t o", p=128, t=2),
                )

            wq_r = wq.rearrange("(b t p) o -> p b t o", p=128, t=2)
            nc.sync.dma_start(out=wq_s[:, :, :, 0:512], in_=wq_r[:, :, :, 0:512])
            nc.sync.dma_start(out=bq_s[:], in_=bq_d[:, :, :])
            nc.sync.dma_start(out=wq_s[:, :, :, 512:D], in_=wq_r[:, :, :, 512:D])
            # xt(0) issued here (below, right after wq) via dma_xt(0) call
            _late_dmas = []

            def late_dmas():
                w_dma(wk_s, wk)
                nc.sync.dma_start(out=bk_s[:], in_=bk_d[:, :, :])
                w_dma(wv_s, wv)
                nc.sync.dma_start(out=bvb_s[:], in_=bvb[:, :])

            if not trivial_gb:
                gb_s = spool.tile([128, D], f32, tag="gb", name="gb")
                bb_s = spool.tile([128, D], f32, tag="bb", name="bb")
            eps_s = spool.tile([128, 1], f32, tag="eps", name="eps")
            nc.vector.memset(eps_s[:], EPS)
            ident_s = spool.tile([128, 128], bf16, tag="ident", name="ident")

            # ---------------- per-graph tile registries ----------------
            xt_t, qT_t, kT_t, v_t, ot_t, xr_t, yg_t = ({} for _ in range(7))
            st_t, pe_t_t, pt_t, av_t, rb_t, rc_t = ({} for _ in range(6))

            def dma_xt(g):
                xt = xpool.tile([128, 4, 2, N], f8, tag="xt", name="xt")
                xt_t[g] = xt
                nc.sync.dma_start(
                    out=xt[:], in_=xT_r[:, :, :, g * N:(g + 1) * N])

            def dma_xr(g):
                xrg = xypool.tile([128, 2, D], bf16, tag="xrg", name="xrg")
                xr_t[g] = xrg
                nc.sync.dma_start(out=xrg[:], in_=xr_r[:, g])

            def dma_eb(a, h):
                # one DMA covers the (h, h+1) head pair
                eb = ebpool.tile([128, 2, 2, N], bf16, tag="eb", name="eb")
                eb_t[(a, h)] = eb
                nc.sync.dma_start(out=eb[:], in_=eb_r[:, a, h:h + 2])

            eb_t = {}

            # ----- stage A: QKV projection units -----
            def a_unit_qk(g, u):
                # u 0..3: Q ob-pair (2u, 2u+1); u 4..7: K ob-pair.
                # The per-feature bias rides in as a 9th fp8 matmul
                # (bias-row x ones-row), so one evac covers both obs.
                if u < 4:
                    w_s, b_s, pr = wq_s, bq_s, u
                    if pr == 0:
                        qT_t[g] = qkpool.tile([128, 8, N], bf16, tag="qT",
                                              name="qT")
                    dst = qT_t[g]
                else:
                    w_s, b_s, pr = wk_s, bk_s, u - 4
                    if pr == 0:
                        kT_t[g] = qkpool.tile([128, 8, N], bf16, tag="kT",
                                              name="kT")
                    dst = kT_t[g]
                xt = xt_t[g]
                ps = pqkpool.tile([128, 2, 256], f32, tag="ps512",
                                  name="ps512")
                for j in range(2):
                    ob = 2 * pr + j
                    for b in range(4):
                        nc.tensor.matmul(
                            ps[:, j, :],
                            w_s[:, b, :, ob * 128:(ob + 1) * 128],
                            xt[:, b, :, :],
                            start=(b == 0), stop=False,
                            perf_mode=DR,
                        )
                    nc.tensor.matmul(
                        ps[:, j, :], b_s[:, ob, :], onesr[:],
                        start=False, stop=True,
                    )
                cg = cs[g]
                nc.scalar.copy(
                    out=dst[:, 2 * pr:2 * pr + 2, 0:cg],
                    in_=ps[:, :, 0:cg],
                )

            def a_unit_v(g, u):
                # u 0..3 -> (jb, oc)
                jb, oc = u // 2, u % 2
                if u == 0:
                    v_t[g] = vpool.tile([128, 2, H, DH + 1], bf16, tag="v", name="v")
                    nc.vector.memset(v_t[g][:, :, :, DH:DH + 1], 1.0)
                xt = xt_t[g]
                ps = pqkpool.tile([128, 2, 256], f32, tag="ps512", name="ps512")
                for b in range(4):
                    nc.tensor.matmul(
                        ps[:, :, :],
                        xt[:, b, :, jb * 128:(jb + 1) * 128],
                        wv_s[:, b, :, oc * 512:(oc + 1) * 512],
                        start=(b == 0), stop=(b == 3),
                        perf_mode=DR,
                    )
                nc.vector.tensor_add(
                    out=v_t[g][:, jb, oc * 8:(oc + 1) * 8, 0:DH],
                    in0=ps[:, :, :],
                    in1=bvb_s[:, oc * 512:(oc + 1) * 512],
                )

            # stage A slot map: V units at iA in (2,6,10,14); QK units fill
            # the rest in order, doubled at iA in (0,5,9,13). Graph 0 runs
            # while weights still stream in, so its V units go last (wv
            # arrives after wq/wk).
            def a_unit(g, iA):
                if g == 0:
                    if iA < 8:
                        a_unit_qk(g, iA)
                    elif iA in (8, 10, 12, 14):
                        a_unit_v(g, (iA - 8) // 2)
                    return
                if iA in (2, 6, 10, 14):
                    a_unit_v(g, (iA - 2) // 4)
                    return
                start = _a_qk_cursor[g]
                if start < 8:
                    a_unit_qk(g, start)
                _a_qk_cursor[g] = start + 1

            _a_qk_cursor = {g: 0 for g in range(GPC)}

            # ----- stage B: attention -----
            def emit_S(a, h):
                hp, ho = h % 2, h // 2
                ca = cs[a]
                st = pstpool.tile([128, 2, N], f32, tag="st", name="st")
                st_t[(a, h)] = st
                qh = qT_t[a][hp * 64:(hp + 1) * 64, ho, 0:ca]
                for jb in range(2):
                    nc.tensor.matmul(
                        st[:, jb, 0:ca],
                        kT_t[a][hp * 64:(hp + 1) * 64, ho,
                                jb * 128:(jb + 1) * 128],
                        qh,
                        start=True, stop=True,
                    )

            def emit_exp(a, h):
                ca = cs[a]
                pe = ptpool.tile([128, 2, N], bf16, tag="pe", name="pe")
                pe_t_t[(a, h)] = pe
                nc.scalar.activation(
                    out=pe[:, :, 0:ca], in_=st_t[(a, h)][:, :, 0:ca],
                    func=AF.Exp,
                )

            def emit_pmul(a, h):
                ca = cs[a]
                pt = ptpool.tile([128, 2, N], bf16, tag="pt", name="pt")
                pt_t[(a, h)] = pt
                nc.gpsimd.tensor_mul(
                    out=pt[:, :, 0:ca], in0=pe_t_t[(a, h)][:, :, 0:ca],
                    in1=eb_t[(a, h - h % 2)][:, h % 2, :, 0:ca],
                )

            def emit_AV(a, h):
                # query-major attention-value product: avT[i, d] with the
                # softmax denominator in column 64 (ones column of V)
                avt = pavpool.tile([128, 2, DH + 1], f32, tag="avt", name="avt")
                av_t[(a, h)] = avt
                pt = pt_t[(a, h)]
                for ib in range(2):
                    for jb in range(2):
                        nc.tensor.matmul(
                            avt[:, ib, :],
                            pt[:, jb, ib * 128:(ib + 1) * 128],
                            v_t[a][:, jb, h, :],
                            start=(jb == 0), stop=(jb == 1),
                        )

            def emit_recip(a, h):
                rc = rcpool.tile([128, 2, 1], f32, tag="rc", name="rc")
                rc_t[(a, h)] = rc
                nc.vector.reciprocal(rc[:], av_t[(a, h)][:, :, DH:DH + 1])

            def emit_scale(a, h):
                # normalize rows (query-major -> per-partition scalar) and
                # evacuate to the query-major attention output
                if h == 0:
                    ot_t[a] = otpool.tile([128, 2, H, DH], bf16, tag="ot",
                                          name="ot")
                avt, rc = av_t[(a, h)], rc_t[(a, h)]
                for ib in range(2):
                    nc.vector.tensor_scalar(
                        out=ot_t[a][:, ib, h, :], in0=avt[:, ib, 0:DH],
                        scalar1=rc[:, ib, :], scalar2=None,
                        op0=mybir.AluOpType.mult,
                    )

            # ----- stage C: out-proj + residual + LN + store -----
            otf_t = {}

            def c_transp(g, ib):
                # feature-major ot for the out-projection: PE-transpose the
                # query-major [i, (h d)] tiles into [(h d), i] blocks
                otr = ptrpool.tile([128, 8, 128], bf16, tag="otr", name="otr")
                for tt in range(8):
                    nc.tensor.matmul(
                        otr[:, tt, :],
                        ot_t[g][:, ib, 2 * tt:2 * tt + 2, :],
                        ident_s[:],
                        is_transpose=True, start=True, stop=True,
                    )
                otf = otpool.tile([128, 8, 128], f8, tag="otf", name="otf")
                otf_t[(g, ib)] = otf
                nc.scalar.copy(out=otf[:], in_=otr[:])

            def c_psy(g, ib, oc):
                if (ib, oc) == (0, 0):
                    yg_t[g] = xypool.tile([128, 2, D], bf16, tag="yg",
                                          name="yg")
                ps = pqkpool.tile([128, 2, 256], f32, tag="ps512",
                                  name="ps512")
                otf = otf_t[(g, ib)]
                for b in range(4):
                    nc.tensor.matmul(
                        ps[:, :, :],
                        otf[:, 2 * b:2 * b + 2, :],
                        wp_s[:, b, :, oc * 512:(oc + 1) * 512],
                        start=(b == 0), stop=(b == 3),
                        perf_mode=DR,
                    )
                nc.vector.tensor_add(
                    out=yg_t[g][:, ib, oc * 512:(oc + 1) * 512],
                    in0=ps[:, :, :],
                    in1=xr_t[g][:, ib, oc * 512:(oc + 1) * 512],
                )

            ln_t = {}

            def c_ln_a(g, ib):
                yt = yg_t[g][:, ib, :]
                stats = lnpool.tile([128, 2, 6], f32, tag="stats", name="stats")
                for sg in range(2):
                    nc.vector.bn_stats(
                        out=stats[:, sg, :], in_=yt[:, sg * 512:(sg + 1) * 512]
                    )
                mv = lnpool.tile([128, 2], f32, tag="mv", name="mv")
                nc.vector.bn_aggr(out=mv[:], in_=stats[:])
                ln_t[(g, ib)] = mv

            def c_ln_b(g, ib):
                yt = yg_t[g][:, ib, :]
                mv = ln_t[(g, ib)]
                std = lnpool.tile([128, 1], f32, tag="std", name="std")
                nc.scalar.activation(
                    out=std[:], in_=mv[:, 1:2], func=AF.Sqrt,
                    bias=eps_s[:], scale=1.0,
                )
                nc.vector.reciprocal(std[:], std[:])
                nc.vector.tensor_scalar(
                    out=yt, in0=yt,
                    scalar1=mv[:, 0:1], scalar2=std[:],
                    op0=mybir.AluOpType.subtract, op1=mybir.AluOpType.mult,
                )
                if not trivial_gb:
                    nc.vector.tensor_mul(out=yt, in0=yt, in1=gb_s[:])
                    nc.vector.tensor_add(out=yt, in0=yt, in1=bb_s[:])

            # NOTE: scale(g, 14) / scale(g, 15) are emitted at iC=0 / 1
            # of this gslot, so the first transpose (which reads the
            # whole ot tile) must come at iC >= 2.
            C_MAP = {2: [(c_transp, 0)], 3: [(c_psy, 0, 0)],
                     4: [(c_transp, 1)], 5: [(c_psy, 0, 1)],
                     7: [(c_ln_a, 0)], 8: [(c_ln_b, 0)],
                     9: [(c_psy, 1, 0)], 11: [(c_psy, 1, 1)],
                     13: [(c_ln_a, 1)], 14: [(c_ln_b, 1)], 15: [("y",)]}
            # last graph: nothing else to pace against -> pack densely
            C_MAP_LAST = {2: [(c_transp, 0)],
                          3: [(c_psy, 0, 0), (c_transp, 1)],
                          4: [(c_psy, 0, 1), (c_psy, 1, 0)],
                          5: [(c_ln_a, 0), (c_psy, 1, 1)],
                          6: [(c_ln_b, 0), (c_ln_a, 1)],
                          7: [(c_ln_b, 1)], 8: [("y",)]}

            def c_unit(g, iC):
                cmap = C_MAP_LAST if g == GPC - 1 else C_MAP
                for u in cmap.get(iC, ()):
                    if u[0] == "y":
                        nc.sync.dma_start(out=y_r[:, g], in_=yg_t[g][:])
                    else:
                        u[0](g, *u[1:])

            # stale-region hygiene: pt columns >= cs and qT/kT key columns
            # >= cs are read by matmuls (results discarded) -- memset the
            # ring buffers once so those reads are finite, never NaN.
            for _tag, _pool, _shape, _dt, _n in (
                    ("qT", qkpool, [128, 8, N], bf16, 2),
                    ("kT", qkpool, [128, 8, N], bf16, 2),
                    ("pt", ptpool, [128, 2, N], bf16, 4)):
                for _i in range(_n):
                    _tl = _pool.tile(_shape, _dt, tag=_tag, name=_tag)
                    nc.gpsimd.memset(_tl[:], 0.0)

            # ---------------- the slot schedule ----------------
            dma_xt(0)
            late_dmas()
            total = (GPC + 2) * 16
            for t in range(total):
                gA, iA = divmod(t, 16)
                if t == 18:
                    w_dma(wp_s, wp)
                    nc.sync.dma_start(out=ident_s[:], in_=ident[:, :])
                    if not trivial_gb:
                        nc.sync.dma_start(out=gb_s[:], in_=gb[:, :])
                        nc.sync.dma_start(out=bb_s[:], in_=bb[:, :])
                # DMA issues first
                if t >= 13:
                    a2, h2 = divmod(t - 13, 16)
                    if a2 < GPC and h2 % 2 == 0:
                        dma_eb(a2, h2)
                if iA == 10 and gA + 1 < GPC:
                    dma_xt(gA + 1)
                if iA == 12 and 0 <= gA - 1 < GPC:
                    dma_xr(gA - 1)
                # stage B: S / exp / pmul for head iB of graph aB
                aB, iB = divmod(t - 16, 16)
                if 0 <= aB < GPC:
                    emit_S(aB, iB)
                    emit_exp(aB, iB)
                    emit_pmul(aB, iB)
                aP, iP = divmod(t - 18, 16)
                # stage A unit first: it is always ready, so the PE queue
                # never stalls ahead of it waiting for the AV dependency
                if gA < GPC:
                    a_unit(gA, iA)
                # stage B completions for the previous head
                if 0 <= aP < GPC:
                    emit_AV(aP, iP)
                    emit_recip(aP, iP)
                # stage B: normalize+evacuate (after the A unit, hiding
                # the reciprocal latency)
                if 0 <= aP < GPC:
                    emit_scale(aP, iP)
                # stage C unit
                gC, iC = divmod(t - 32, 16)
                if 0 <= gC < GPC:
                    c_unit(gC, iC)

    _split_excess_waits(nc)
    _CACHE[key] = nc
    return nc


def make_in_maps(x, batch, attn_bias, Wq, bq, Wk, bk, Wv, bv, Wp, bp,
                 gamma, beta):
    """Host-side shard prep. Returns (in_maps, trivial_gb, batch, pos)."""
    from ml_dtypes import bfloat16, float8_e4m3

    x = np.asarray(x, np.float32)
    batch = np.asarray(batch, np.int32)
    attn_bias = np.asarray(attn_bias, np.float32)
    T = x.shape[0]

    counts = np.bincount(batch, minlength=B)
    offsets = np.zeros(B, np.int64)
    np.cumsum(counts[:-1], out=offsets[1:])
    pos = np.arange(T, dtype=np.int64) - offsets[batch]

    # count-sorted placement: rank r -> core r%8, slot r//8, so every slot
    # holds graphs of similar size and one shared per-slot trim bound works
    # across cores
    order = np.argsort(-counts, kind="stable")
    cs = tuple(int(counts[order[s * NCORES]]) for s in range(GPC))

    xd = np.zeros((B, N, D), np.float32)
    xd[batch, pos] = x
    xr_full = xd + np.asarray(bp, np.float32)  # residual + out-proj bias

    # exp of transposed bias with masked (padded) keys zeroed
    ebT = np.exp(attn_bias.transpose(0, 1, 3, 2)).astype(np.float32)
    for b in range(B):
        if counts[b] < N:
            ebT[b, :, counts[b]:, :] = 0.0
    ebT = ebT.reshape(B, H, 2, 128, N).astype(bfloat16)

    wq_h = np.ascontiguousarray(
        (np.asarray(Wq, np.float32).T * SCALE)).astype(float8_e4m3)
    wk_h = np.ascontiguousarray(np.asarray(Wk, np.float32).T).astype(float8_e4m3)
    wv_h = np.ascontiguousarray(np.asarray(Wv, np.float32).T).astype(float8_e4m3)
    wp_h = np.ascontiguousarray(np.asarray(Wp, np.float32).T).astype(float8_e4m3)
    bq_h = np.ascontiguousarray(
        (np.asarray(bq, np.float32) * SCALE).reshape(1, 8, 128)
    ).astype(float8_e4m3)
    bk_h = np.ascontiguousarray(
        np.asarray(bk, np.float32).reshape(1, 8, 128)).astype(float8_e4m3)
    bv_h = np.broadcast_to(np.asarray(bv, np.float32), (128, D)).copy()
    gamma = np.asarray(gamma, np.float32)
    beta = np.asarray(beta, np.float32)
    trivial_gb = bool(np.all(gamma == 1.0) and np.all(beta == 0.0))
    g_h = np.broadcast_to(gamma, (128, D)).copy()
    b_h = np.broadcast_to(beta, (128, D)).copy()
    ident_h = np.eye(128, dtype=np.float32).astype(bfloat16)

    in_maps = []
    for c in range(NCORES):
        gidx = order[c::NCORES]  # slot s -> graph order[s*8 + c]
        xc = xd[gidx].reshape(RPC, D)
        in_maps.append({
            "xT": np.ascontiguousarray(xc.T).astype(float8_e4m3),
            "xr": np.ascontiguousarray(
                xr_full[gidx].reshape(RPC, D)).astype(bfloat16),
            "ebias": np.ascontiguousarray(ebT[gidx]),
            "wq": wq_h, "wk": wk_h, "wv": wv_h, "wp": wp_h,
            "bq_d": bq_h, "bk_d": bk_h, "bvb": bv_h, "gb": g_h, "bb": b_h,
            "ident": ident_h,
        })
    return in_maps, trivial_gb, cs, order, batch, pos


def unshard(results, order, batch, pos):
    yc = np.stack(
        [np.asarray(res["y"], np.float32).reshape(GPC, N, D)
         for res in results])                       # [core, slot, N, D]
    yd = np.empty((B, N, D), np.float32)
    for r, g in enumerate(order):
        yd[g] = yc[r % NCORES, r // NCORES]
    return np.ascontiguousarray(yd[batch, pos])


def kernel(**inputs) -> np.ndarray:
    in_maps, trivial_gb, cs, order, batch, pos = make_in_maps(**inputs)
    nc = build_program(trivial_gb, cs)
    from concourse.bass_utils import run_bass_kernel_spmd

    res = run_bass_kernel_spmd(nc, in_maps, core_ids=list(range(NCORES)))
    return unshard(res.results, order, batch, pos)


# revision 7
# speedup vs baseline: 1.0127x; 1.0127x over previous
"""Biased self-attention layer (graph-batched) on 8 Trainium2 NeuronCores.

Data-parallel over B=64 graphs, 8 graphs/core, dense-padded to N=256.

Design (vs the 570us baseline):
- single fused pass per graph (QKV -> attention -> out-proj -> LN), no
  DRAM scratch roundtrip;
- fp8(E4M3) DoubleRow matmuls for the QKV and output projections (2
  contraction rows/cycle), with the q/k biases folded in as a 9th
  ones-row matmul so each paired Q/K evacuation is a single plain copy;
  bf16 operands for S / AV; f32 PSUM accumulation throughout;
- softmax as exp(S) * exp(bias) with host-precomputed exp-bias (masked
  keys become exact 0); the bias multiply runs on the otherwise-idle
  GPSIMD engine, kept dedicated to it (the exp->pmul->AV chain is
  latency-critical, so no bulk work goes on the Act or Pool queues);
- query-major AV with the softmax denominator as a ones-column of V, so
  normalization is one per-partition tensor_scalar per head-half, then a
  PE transpose restores feature-major layout for the projection;
- graphs count-sorted across cores so one shared SPMD program can trim
  all per-head softmax work to a per-slot valid-query bound;
- work emitted on a software-pipelined slot schedule (3 graph stages in
  flight: QKV(g), attention(g-1), out-proj/LN(g-2)), A-units emitted
  ahead of cross-engine completions so the PE queue never head-of-line
  blocks on them.
"""

import sys

for _p in ("/opt/pypackages", "/opt/trn_rl_repo"):
    if _p not in sys.path:
        sys.path.insert(0, _p)

import numpy as np

B, N, H, D = 64, 256, 16, 1024
DH = D // H
SCALE = DH ** -0.5
EPS = 1e-5
NCORES = 8
GPC = B // NCORES          # graphs per core
RPC = GPC * N              # padded rows per core (2048)

_CACHE = {}


def _split_excess_waits(nc, maxw=1):
    """This walrus build accepts at most one sync-wait per instruction
    (TPB_CTRL and S3_LW structs reject more). Move excess waits onto
    preceding single-wait NOPs on the same engine queue."""
    import bass_rust

    fn = nc.m.functions[0]
    for bb in fn.blocks:
        insts = bb.instructions
        out = []
        for inst in list(insts):
            si = inst.sync_info
            if si is not None and si.on_wait and len(si.on_wait) > maxw:
                waits = list(si.on_wait)
                si.on_wait = waits[-maxw:]
                for w in waits[:-maxw]:
                    nop_bi = nc.engines[inst.engine].nop(
                        nofuse=True, hint="wait_split"
                    )
                    ni = nop_bi.ins
                    cur = nc.cur_bb.bb.instructions
                    assert cur[-1].name == ni.name
                    cur.pop()
                    nsi = ni.sync_info
                    if nsi is None:
                        ni.sync_info = bass_rust.SyncInfo(
                            on_wait=[w], on_update=[]
                        )
                    else:
                        nsi.on_wait = [w]
                    out.append(ni)
            out.append(inst)
        if len(out) != len(insts):
            insts[:] = out


def build_program(trivial_gb=True, cs=(N,) * GPC):
    """Build the SPMD Bass program. cs[s] = max valid node count of the
    graphs placed in slot s (graphs are count-sorted on the host so all
    cores share one program); per-head softmax work is trimmed to cs[s]
    query columns, with out-of-range columns left stale-but-finite and
    discarded by the host gather."""
    key = ("nc", trivial_gb, tuple(cs))
    if key in _CACHE:
        return _CACHE[key]

    import concourse.bass as bass
    import concourse.mybir as mybir
    import concourse.tile as tile

    dt = mybir.dt
    AF = mybir.ActivationFunctionType
    f32 = dt.float32
    f32r = dt.float32r
    bf16 = dt.bfloat16

    nc = bass.Bass("TRN2", target_bir_lowering=False, debug=False)

    f8 = dt.float8e4
    DR = mybir.MatmulPerfMode.DoubleRow
    xT = nc.dram_tensor("xT", [D, RPC], f8, kind="ExternalInput")
    xr = nc.dram_tensor("xr", [RPC, D], bf16, kind="ExternalInput")  # x + bp
    ebias = nc.dram_tensor("ebias", [GPC, H, 2, 128, N], bf16,
                           kind="ExternalInput")  # exp(bias^T), masked=0
    wq = nc.dram_tensor("wq", [D, D], f8, kind="ExternalInput")  # Wq.T*scale
    wk = nc.dram_tensor("wk", [D, D], f8, kind="ExternalInput")
    wv = nc.dram_tensor("wv", [D, D], f8, kind="ExternalInput")
    wp = nc.dram_tensor("wp", [D, D], f8, kind="ExternalInput")
    bq_d = nc.dram_tensor("bq_d", [1, 8, 128], f8, kind="ExternalInput")
    bk_d = nc.dram_tensor("bk_d", [1, 8, 128], f8, kind="ExternalInput")
    bvb = nc.dram_tensor("bvb", [128, D], f32, kind="ExternalInput")
    gb = nc.dram_tensor("gb", [128, D], f32, kind="ExternalInput")
    bb = nc.dram_tensor("bb", [128, D], f32, kind="ExternalInput")
    ident = nc.dram_tensor("ident", [128, 128], bf16, kind="ExternalInput")
    y = nc.dram_tensor("y", [RPC, D], bf16, kind="ExternalOutput")

    xT_r = xT.rearrange("(b t p) r -> p b t r", p=128, t=2)
    eb_r = ebias.rearrange("g h jb p i -> p g h jb i")
    xr_r = xr.rearrange("(g ih p) o -> p g ih o", p=128, ih=2)
    y_r = y.rearrange("(g ih p) o -> p g ih o", p=128, ih=2)

    with tile.TileContext(nc) as tc:
        with tc.tile_pool(name="w", bufs=1) as wpool, \
             tc.tile_pool(name="sm", bufs=1) as spool, \
             tc.tile_pool(name="x", bufs=2) as xpool, \
             tc.tile_pool(name="qk", bufs=2) as qkpool, \
             tc.tile_pool(name="v", bufs=2) as vpool, \
             tc.tile_pool(name="eb", bufs=6) as ebpool, \
             tc.tile_pool(name="pt", bufs=4) as ptpool, \
             tc.tile_pool(name="rc", bufs=4) as rcpool, \
             tc.tile_pool(name="ot", bufs=2) as otpool, \
             tc.tile_pool(name="xy", bufs=2) as xypool, \
             tc.tile_pool(name="ln", bufs=4) as lnpool, \
             tc.tile_pool(name="pqk", bufs=3, space="PSUM") as pqkpool, \
             tc.tile_pool(name="pst", bufs=2, space="PSUM") as pstpool, \
             tc.tile_pool(name="pav", bufs=2, space="PSUM") as pavpool, \
             tc.tile_pool(name="ptr", bufs=1, space="PSUM") as ptrpool:

            # ---------------- resident weights / constants ----------------
            # DMA order matters at startup: the first Q matmuls need only
            # wq + x(0) + bq, so issue those first; K/V/P weights stream in
            # while Q projections already run.
            wq_s = wpool.tile([128, 4, 2, D], f8, tag="wq", name="wq")
            wk_s = wpool.tile([128, 4, 2, D], f8, tag="wk", name="wk")
            wv_s = wpool.tile([128, 4, 2, D], f8, tag="wv", name="wv")
            wp_s = wpool.tile([128, 4, 2, D], f8, tag="wp", name="wp")
            bq_s = spool.tile([1, 8, 128], f8, tag="bq", name="bq")
            bk_s = spool.tile([1, 8, 128], f8, tag="bk", name="bk")
            onesr = spool.tile([1, N], f8, tag="onesr", name="onesr")
            nc.vector.memset(onesr[:], 1.0)
            bvb_s = spool.tile([128, D], f32, tag="bvb", name="bvb")

            def w_dma(w_s, w_d):
                nc.sync.dma_start(
                    out=w_s[:],
                    in_=w_d.rearrange("(b t p) o -> p b t o", p=128, t=2),
                )

            wq_r = wq.rearrange("(b t p) o -> p b t o", p=128, t=2)
            nc.sync.dma_start(out=wq_s[:, :, :, 0:512], in_=wq_r[:, :, :, 0:512])
            nc.sync.dma_start(out=bq_s[:], in_=bq_d[:, :, :])
            nc.sync.dma_start(out=wq_s[:, :, :, 512:D], in_=wq_r[:, :, :, 512:D])
            # xt(0) issued here (below, right after wq) via dma_xt(0) call
            _late_dmas = []

            def late_dmas():
                w_dma(wk_s, wk)
                nc.sync.dma_start(out=bk_s[:], in_=bk_d[:, :, :])
                w_dma(wv_s, wv)
                nc.sync.dma_start(out=bvb_s[:], in_=bvb[:, :])

            if not trivial_gb:
                gb_s = spool.tile([128, D], f32, tag="gb", name="gb")
                bb_s = spool.tile([128, D], f32, tag="bb", name="bb")
            eps_s = spool.tile([128, 1], f32, tag="eps", name="eps")
            nc.vector.memset(eps_s[:], EPS)
            ident_s = spool.tile([128, 128], bf16, tag="ident", name="ident")

            # ---------------- per-graph tile registries ----------------
            xt_t, qT_t, kT_t, v_t, ot_t, xr_t, yg_t = ({} for _ in range(7))
            st_t, pe_t_t, pt_t, av_t, rb_t, rc_t = ({} for _ in range(6))

            def dma_xt(g):
                xt = xpool.tile([128, 4, 2, N], f8, tag="xt", name="xt")
                xt_t[g] = xt
                nc.sync.dma_start(
                    out=xt[:], in_=xT_r[:, :, :, g * N:(g + 1) * N])

            def dma_xr(g):
                xrg = xypool.tile([128, 2, D], bf16, tag="xrg", name="xrg")
                xr_t[g] = xrg
                nc.sync.dma_start(out=xrg[:], in_=xr_r[:, g])

            def dma_eb(a, h):
                # one DMA covers the (h, h+1) head pair
                eb = ebpool.tile([128, 2, 2, N], bf16, tag="eb", name="eb")
                eb_t[(a, h)] = eb
                nc.sync.dma_start(out=eb[:], in_=eb_r[:, a, h:h + 2])

            eb_t = {}

            # ----- stage A: QKV projection units -----
            def a_unit_qk(g, u):
                # u 0..3: Q ob-pair (2u, 2u+1); u 4..7: K ob-pair.
                # The per-feature bias rides in as a 9th fp8 matmul
                # (bias-row x ones-row), so one evac covers both obs.
                if u < 4:
                    w_s, b_s, pr = wq_s, bq_s, u
                    if pr == 0:
                        qT_t[g] = qkpool.tile([128, 8, N], bf16, tag="qT",
                                              name="qT")
                    dst = qT_t[g]
                else:
                    w_s, b_s, pr = wk_s, bk_s, u - 4
                    if pr == 0:
                        kT_t[g] = qkpool.tile([128, 8, N], bf16, tag="kT",
                                              name="kT")
                    dst = kT_t[g]
                xt = xt_t[g]
                ps = pqkpool.tile([128, 2, 256], f32, tag="ps512",
                                  name="ps512")
                for j in range(2):
                    ob = 2 * pr + j
                    for b in range(4):
                        nc.tensor.matmul(
                            ps[:, j, :],
                            w_s[:, b, :, ob * 128:(ob + 1) * 128],
                            xt[:, b, :, :],
                            start=(b == 0), stop=False,
                            perf_mode=DR,
                        )
                    nc.tensor.matmul(
                        ps[:, j, :], b_s[:, ob, :], onesr[:],
                        start=False, stop=True,
                    )
                cg = cs[g]
                nc.scalar.copy(
                    out=dst[:, 2 * pr:2 * pr + 2, 0:cg],
                    in_=ps[:, :, 0:cg],
                )

            def a_unit_v(g, u):
                # u 0..3 -> (jb, oc)
                jb, oc = u // 2, u % 2
                if u == 0:
                    v_t[g] = vpool.tile([128, 2, H, DH + 1], bf16, tag="v", name="v")
                    nc.vector.memset(v_t[g][:, :, :, DH:DH + 1], 1.0)
                xt = xt_t[g]
                ps = pqkpool.tile([128, 2, 256], f32, tag="ps512", name="ps512")
                for b in range(4):
                    nc.tensor.matmul(
                        ps[:, :, :],
                        xt[:, b, :, jb * 128:(jb + 1) * 128],
                        wv_s[:, b, :, oc * 512:(oc + 1) * 512],
                        start=(b == 0), stop=(b == 3),
                        perf_mode=DR,
                    )
                nc.vector.tensor_add(
                    out=v_t[g][:, jb, oc * 8:(oc + 1) * 8, 0:DH],
                    in0=ps[:, :, :],
                    in1=bvb_s[:, oc * 512:(oc + 1) * 512],
                )

            # stage A slot map: V units at iA in (2,6,10,14); QK units fill
            # the rest in order, doubled at iA in (0,5,9,13). Graph 0 runs
            # while weights still stream in, so its V units go last (wv
            # arrives after wq/wk).
            def a_unit(g, iA):
                if g == 0:
                    if iA < 8:
                        a_unit_qk(g, iA)
                    elif iA in (8, 10, 12, 14):
                        a_unit_v(g, (iA - 8) // 2)
                    return
                if iA in (2, 6, 10, 14):
                    a_unit_v(g, (iA - 2) // 4)
                    return
                start = _a_qk_cursor[g]
                if start < 8:
                    a_unit_qk(g, start)
                _a_qk_cursor[g] = start + 1

            _a_qk_cursor = {g: 0 for g in range(GPC)}

            # ----- stage B: attention -----
            def emit_S(a, h):
                hp, ho = h % 2, h // 2
                ca = cs[a]
                st = pstpool.tile([128, 2, N], f32, tag="st", name="st")
                st_t[(a, h)] = st
                qh = qT_t[a][hp * 64:(hp + 1) * 64, ho, 0:ca]
                for jb in range(2):
                    nc.tensor.matmul(
                        st[:, jb, 0:ca],
                        kT_t[a][hp * 64:(hp + 1) * 64, ho,
                                jb * 128:(jb + 1) * 128],
                        qh,
                        start=True, stop=True,
                    )

            def emit_exp(a, h):
                ca = cs[a]
                pe = ptpool.tile([128, 2, N], bf16, tag="pe", name="pe")
                pe_t_t[(a, h)] = pe
                nc.scalar.activation(
                    out=pe[:, :, 0:ca], in_=st_t[(a, h)][:, :, 0:ca],
                    func=AF.Exp,
                )

            def emit_pmul(a, h):
                ca = cs[a]
                pt = ptpool.tile([128, 2, N], bf16, tag="pt", name="pt")
                pt_t[(a, h)] = pt
                nc.gpsimd.tensor_mul(
                    out=pt[:, :, 0:ca], in0=pe_t_t[(a, h)][:, :, 0:ca],
                    in1=eb_t[(a, h - h % 2)][:, h % 2, :, 0:ca],
                )

            def emit_AV(a, h):
                # query-major attention-value product: avT[i, d] with the
                # softmax denominator in column 64 (ones column of V)
                avt = pavpool.tile([128, 2, DH + 1], f32, tag="avt", name="avt")
                av_t[(a, h)] = avt
                pt = pt_t[(a, h)]
                for ib in range(2):
                    for jb in range(2):
                        nc.tensor.matmul(
                            avt[:, ib, :],
                            pt[:, jb, ib * 128:(ib + 1) * 128],
                            v_t[a][:, jb, h, :],
                            start=(jb == 0), stop=(jb == 1),
                        )

            def emit_recip(a, h):
                rc = rcpool.tile([128, 2, 1], f32, tag="rc", name="rc")
                rc_t[(a, h)] = rc
                nc.vector.reciprocal(rc[:], av_t[(a, h)][:, :, DH:DH + 1])

            def emit_scale(a, h):
                # normalize rows (query-major -> per-partition scalar) and
                # evacuate to the query-major attention output
                if h == 0:
                    ot_t[a] = otpool.tile([128, 2, H, DH], bf16, tag="ot",
                                          name="ot")
                avt, rc = av_t[(a, h)], rc_t[(a, h)]
                for ib in range(2):
                    nc.vector.tensor_scalar(
                        out=ot_t[a][:, ib, h, :], in0=avt[:, ib, 0:DH],
                        scalar1=rc[:, ib, :], scalar2=None,
                        op0=mybir.AluOpType.mult,
                    )

            # ----- stage C: out-proj + residual + LN + store -----
            otf_t = {}

            def c_transp(g, ib):
                # feature-major ot for the out-projection: PE-transpose the
                # query-major [i, (h d)] tiles into [(h d), i] blocks
                otr = ptrpool.tile([128, 8, 128], bf16, tag="otr", name="otr")
                for tt in range(8):
                    nc.tensor.matmul(
                        otr[:, tt, :],
                        ot_t[g][:, ib, 2 * tt:2 * tt + 2, :],
                        ident_s[:],
                        is_transpose=True, start=True, stop=True,
                    )
                otf = otpool.tile([128, 8, 128], f8, tag="otf", name="otf")
                otf_t[(g, ib)] = otf
                nc.scalar.copy(out=otf[:], in_=otr[:])

            def c_psy(g, ib, oc):
                if (ib, oc) == (0, 0):
                    yg_t[g] = xypool.tile([128, 2, D], bf16, tag="yg",
                                          name="yg")
                ps = pqkpool.tile([128, 2, 256], f32, tag="ps512",
                                  name="ps512")
                otf = otf_t[(g, ib)]
                for b in range(4):
                    nc.tensor.matmul(
                        ps[:, :, :],
                        otf[:, 2 * b:2 * b + 2, :],
                        wp_s[:, b, :, oc * 512:(oc + 1) * 512],
                        start=(b == 0), stop=(b == 3),
                        perf_mode=DR,
                    )
                nc.vector.tensor_add(
                    out=yg_t[g][:, ib, oc * 512:(oc + 1) * 512],
                    in0=ps[:, :, :],
                    in1=xr_t[g][:, ib, oc * 512:(oc + 1) * 512],
                )

            ln_t = {}

            def c_ln_a(g, ib):
                yt = yg_t[g][:, ib, :]
                stats = lnpool.tile([128, 2, 6], f32, tag="stats", name="stats")
                for sg in range(2):
                    nc.vector.bn_stats(
                        out=stats[:, sg, :], in_=yt[:, sg * 512:(sg + 1) * 512]
                    )
                mv = lnpool.tile([128, 2], f32, tag="mv", name="mv")
                nc.vector.bn_aggr(out=mv[:], in_=stats[:])
                ln_t[(g, ib)] = mv

            def c_ln_b(g, ib):
                yt = yg_t[g][:, ib, :]
                mv = ln_t[(g, ib)]
                std = lnpool.tile([128, 1], f32, tag="std", name="std")
                nc.scalar.activation(
                    out=std[:], in_=mv[:, 1:2], func=AF.Sqrt,
                    bias=eps_s[:], scale=1.0,
                )
                nc.vector.reciprocal(std[:], std[:])
                nc.vector.tensor_scalar(
                    out=yt, in0=yt,
                    scalar1=mv[:, 0:1], scalar2=std[:],
                    op0=mybir.AluOpType.subtract, op1=mybir.AluOpType.mult,
                )
                if not trivial_gb:
                    nc.vector.tensor_mul(out=yt, in0=yt, in1=gb_s[:])
                    nc.vector.tensor_add(out=yt, in0=yt, in1=bb_s[:])

            # NOTE: scale(g, 14) / scale(g, 15) are emitted at iC=0 / 1
            # of this gslot, so the first transpose (which reads the
            # whole ot tile) must come at iC >= 2.
            C_MAP = {2: [(c_transp, 0)], 3: [(c_psy, 0, 0)],
                     4: [(c_transp, 1)], 5: [(c_psy, 0, 1)],
                     7: [(c_ln_a, 0)], 8: [(c_ln_b, 0)],
                     9: [(c_psy, 1, 0)], 11: [(c_psy, 1, 1)],
                     13: [(c_ln_a, 1)], 14: [(c_ln_b, 1)], 15: [("y",)]}
            # last graph: nothing else to pace against -> pack densely
            C_MAP_LAST = {2: [(c_transp, 0)],
                          3: [(c_psy, 0, 0), (c_transp, 1)],
                          4: [(c_psy, 0, 1), (c_psy, 1, 0)],
                          5: [(c_ln_a, 0), (c_psy, 1, 1)],
                          6: [(c_ln_b, 0), (c_ln_a, 1)],
                          7: [(c_ln_b, 1)], 8: [("y",)]}

            def c_unit(g, iC):
                cmap = C_MAP_LAST if g == GPC - 1 else C_MAP
                for u in cmap.get(iC, ()):
                    if u[0] == "y":
                        nc.sync.dma_start(out=y_r[:, g], in_=yg_t[g][:])
                    else:
                        u[0](g, *u[1:])

            # stale-region hygiene: pt columns >= cs and qT/kT key columns
            # >= cs are read by matmuls (results discarded) -- memset the
            # ring buffers once so those reads are finite, never NaN.
            for _tag, _pool, _shape, _dt, _n in (
                    ("qT", qkpool, [128, 8, N], bf16, 2),
                    ("kT", qkpool, [128, 8, N], bf16, 2),
                    ("pt", ptpool, [128, 2, N], bf16, 4)):
                for _i in range(_n):
                    _tl = _pool.tile(_shape, _dt, tag=_tag, name=_tag)
                    nc.gpsimd.memset(_tl[:], 0.0)

            # ---------------- the slot schedule ----------------
            dma_xt(0)
            late_dmas()
            total = (GPC + 2) * 16
            for t in range(total):
                gA, iA = divmod(t, 16)
                if t == 18:
                    w_dma(wp_s, wp)
                    nc.sync.dma_start(out=ident_s[:], in_=ident[:, :])
                    if not trivial_gb:
                        nc.sync.dma_start(out=gb_s[:], in_=gb[:, :])
                        nc.sync.dma_start(out=bb_s[:], in_=bb[:, :])
                # DMA issues first
                if t >= 13:
                    a2, h2 = divmod(t - 13, 16)
                    if a2 < GPC and h2 % 2 == 0:
                        dma_eb(a2, h2)
                if iA == 10 and gA + 1 < GPC:
                    dma_xt(gA + 1)
                if iA == 12 and 0 <= gA - 1 < GPC:
                    dma_xr(gA - 1)
                # stage B: S / exp / pmul for head iB of graph aB
                aB, iB = divmod(t - 16, 16)
                if 0 <= aB < GPC:
                    emit_S(aB, iB)
                    emit_exp(aB, iB)
                    emit_pmul(aB, iB)
                aP, iP = divmod(t - 18, 16)
                # stage A unit first: it is always ready, so the PE queue
                # never stalls ahead of it waiting for the AV dependency
                if gA < GPC:
                    a_unit(gA, iA)
                # stage B completions for the previous head
                if 0 <= aP < GPC:
                    emit_AV(aP, iP)
                    emit_recip(aP, iP)
                # stage B: normalize+evacuate (after the A unit, hiding
                # the reciprocal latency)
                if 0 <= aP < GPC:
                    emit_scale(aP, iP)
                # stage C unit
                gC, iC = divmod(t - 32, 16)
                if 0 <= gC < GPC:
                    c_unit(gC, iC)

    _split_excess_waits(nc)
    _CACHE[key] = nc
    return nc


def make_in_maps(x, batch, attn_bias, Wq, bq, Wk, bk, Wv, bv, Wp, bp,
                 gamma, beta):
    """Host-side shard prep. Returns (in_maps, trivial_gb, batch, pos)."""
    from ml_dtypes import bfloat16, float8_e4m3

    x = np.asarray(x, np.float32)
    batch = np.asarray(batch, np.int32)
    attn_bias = np.asarray(attn_bias, np.float32)
    T = x.shape[0]

    counts = np.bincount(batch, minlength=B)
    offsets = np.zeros(B, np.int64)
    np.cumsum(counts[:-1], out=offsets[1:])
    pos = np.arange(T, dtype=np.int64) - offsets[batch]

    # count-sorted placement: rank r -> core r%8, slot r//8, so every slot
    # holds graphs of similar size and one shared per-slot trim bound works
    # across cores
    order = np.argsort(-counts, kind="stable")
    cs = tuple(int(counts[order[s * NCORES]]) for s in range(GPC))

    xd = np.zeros((B, N, D), np.float32)
    xd[batch, pos] = x
    xr_full = xd + np.asarray(bp, np.float32)  # residual + out-proj bias

    # exp of transposed bias with masked (padded) keys zeroed
    ebT = np.exp(attn_bias.transpose(0, 1, 3, 2)).astype(np.float32)
    for b in range(B):
        if counts[b] < N:
            ebT[b, :, counts[b]:, :] = 0.0
    ebT = ebT.reshape(B, H, 2, 128, N).astype(bfloat16)

    wq_h = np.ascontiguousarray(
        (np.asarray(Wq, np.float32).T * SCALE)).astype(float8_e4m3)
    wk_h = np.ascontiguousarray(np.asarray(Wk, np.float32).T).astype(float8_e4m3)
    wv_h = np.ascontiguousarray(np.asarray(Wv, np.float32).T).astype(float8_e4m3)
    wp_h = np.ascontiguousarray(np.asarray(Wp, np.float32).T).astype(float8_e4m3)
    bq_h = np.ascontiguousarray(
        (np.asarray(bq, np.float32) * SCALE).reshape(1, 8, 128)
    ).astype(float8_e4m3)
    bk_h = np.ascontiguousarray(
        np.asarray(bk, np.float32).reshape(1, 8, 128)).astype(float8_e4m3)
    bv_h = np.broadcast_to(np.asarray(bv, np.float32), (128, D)).copy()
    gamma = np.asarray(gamma, np.float32)
    beta = np.asarray(beta, np.float32)
    trivial_gb = bool(np.all(gamma == 1.0) and np.all(beta == 0.0))
    g_h = np.broadcast_to(gamma, (128, D)).copy()
    b_h = np.broadcast_to(beta, (128, D)).copy()
    ident_h = np.eye(128, dtype=np.float32).astype(bfloat16)

    in_maps = []
    for c in range(NCORES):
        gidx = order[c::NCORES]  # slot s -> graph order[s*8 + c]
        xc = xd[gidx].reshape(RPC, D)
        in_maps.append({
            "xT": np.ascontiguousarray(xc.T).astype(float8_e4m3),
            "xr": np.ascontiguousarray(
                xr_full[gidx].reshape(RPC, D)).astype(bfloat16),
            "ebias": np.ascontiguousarray(ebT[gidx]),
            "wq": wq_h, "wk": wk_h, "wv": wv_h, "wp": wp_h,
            "bq_d": bq_h, "bk_d": bk_h, "bvb": bv_h, "gb": g_h, "bb": b_h,
            "ident": ident_h,
        })
    return in_maps, trivial_gb, cs, order, batch, pos


def unshard(results, order, batch, pos):
    yc = np.stack(
        [np.asarray(res["y"], np.float32).reshape(GPC, N, D)
         for res in results])                       # [core, slot, N, D]
    yd = np.empty((B, N, D), np.float32)
    for r, g in enumerate(order):
        yd[g] = yc[r % NCORES, r // NCORES]
    return np.ascontiguousarray(yd[batch, pos])


def kernel(**inputs) -> np.ndarray:
    in_maps, trivial_gb, cs, order, batch, pos = make_in_maps(**inputs)
    nc = build_program(trivial_gb, cs)
    from concourse.bass_utils import run_bass_kernel_spmd

    res = run_bass_kernel_spmd(nc, in_maps, core_ids=list(range(NCORES)))
    return unshard(res.results, order, batch, pos)
